# revision 21
# baseline (speedup 1.0000x reference)
"""MoE layer (top-2 of 8 experts), load-balanced expert-parallel across 8
Trainium2 NeuronCores.

Strategy (self-contained; shapes hardcoded for B=4,T=1024,D=1024,E=8,K=2,H=4096):
  - Host: gate logits + top-2 + softmax; solve a small packing problem that
    assigns token groups to a uniform per-core slot template (S=3 slot sizes
    shared by all cores, each slot bound to one expert per core via the DRAM
    input contents).  All 8 cores run the SAME instruction stream; only the
    data (which expert's weights / which tokens) differs, so total PE columns
    per core drops from max-expert-load (1129 for the seed-0 input) to
    sum(slot_sizes) (~1033) -- near the perfect-balance floor of 1024.
  - Device, SPMD over 8 cores: per slot s a transposed-layout FFN
      hT_s = gelu(w1_s.T @ xT_s + b1_s)   [H, N_s]
      yT_s = w2_s.T @ hT_s + b2_s         [D, N_s]
    bf16 matmul inputs, f32 PSUM accumulation.  Slot sizes are all in
    [260, 512]: >=257 keeps LDWEIGHTS hidden under the matmul stream,
    <=512 keeps one PSUM bank per accumulation group.
  - w1 is streamed in multi-m-tile chunks (8KB/partition descriptors) to
    keep the DMA descriptor count flat despite 3x weight traffic.
"""

import os

import numpy as np
import ml_dtypes

B, T, D = 4, 1024, 1024
E, K, H = 8, 2, 4 * 1024
N = B * T
P = 128
KD = D // P           # 8  k-tiles in GEMM1 / output d-tiles in GEMM2
MH = H // P           # 32 m-tiles in GEMM1 / k-tiles in GEMM2
S = 3                 # slots per core
BF16 = ml_dtypes.bfloat16

# w1 chunk plan (m-tiles per DMA): small first chunks so the first PSUM
# group's dependencies land early; 4-tile (8KB/partition) chunks after.
CHUNK_PLAN = [1, 1, 3, 3, 3, 3, 3, 3, 3, 3, 3, 3]
assert sum(CHUNK_PLAN) == MH

LAST_EXEC_TIME_NS = None
_cached_nc = {}


def _solve_template(loads):
    """Pick slot sizes (a>=b>=c, even, in [258,512]) minimizing
    a+b+c such that the 8 copies of each size can cover every expert's load.

    Returns (sizes, n) where n[e] = (n_a, n_b, n_c) slot counts for expert e.
    """
    import math
    loads = [int(x) for x in loads]
    nE = len(loads)

    def feasible(a, b, c):
        sizes = (a, b, c)
        opts = []
        for Le in loads:
            cand = []
            for na in range(0, 9):
                for nb in range(0, 9):
                    rem = Le - na * a - nb * b
                    ncc = max(0, math.ceil(rem / c)) if rem > 0 else 0
                    if ncc > 8:
                        continue
                    waste = na * a + nb * b + ncc * c - Le
                    if waste > 3 * 512:
                        continue
                    cand.append((na, nb, ncc, waste))
            if not cand:
                return None
            cand.sort(key=lambda t: t[3])
            opts.append(cand[:48])
        states = {(0, 0, 0): []}
        for cand in opts:
            new = {}
            for (ua, ub, uc), asg in states.items():
                for (na, nb, ncc, w) in cand:
                    k = (ua + na, ub + nb, uc + ncc)
                    if k[0] > 8 or k[1] > 8 or k[2] > 8:
                        continue
                    if k not in new:
                        new[k] = asg + [(na, nb, ncc)]
            states = new
            if not states:
                return None
        for asg in states.values():
            return asg
        return None

    for Tt in range(1024, 1540, 2):
        for c in range(258, Tt // 3 + 1, 2):
            for bb in range(c, (Tt - c) // 2 + 1, 2):
                a = Tt - bb - c
                if a < bb or a > 512:
                    continue
                asg = feasible(a, bb, c)
                if asg:
                    return (a, bb, c), asg
    # Fallback: one expert per core, 3 equal slots covering max load.
    Cmax = max(max(loads), 3 * 260)
    s = -(-Cmax // 12) * 4
    sizes = (s, s, s)
    asg = [(1, 1, 1) for _ in range(nE)]
    return sizes, asg


def _plan(loads, sels):
    """Build per-core slot plan: core -> [(expert, token_idx_array), ...]."""
    sizes, asg = _solve_template(loads)
    # slot lists per size class: experts with multiplicity
    slot_experts = [[], [], []]
    for e in range(E):
        for cls in range(3):
            slot_experts[cls] += [e] * asg[e][cls]
    for cls in range(3):
        while len(slot_experts[cls]) < E:
            slot_experts[cls].append(0)   # unused slot, zero tokens
        slot_experts[cls] = slot_experts[cls][:E]
    # distribute each expert's tokens across its slots greedily
    offs = [0] * E
    plan = [[None] * S for _ in range(E)]
    for cls in range(3):
        sz = sizes[cls]
        for core in range(E):
            e = slot_experts[cls][core]
            take = min(sz, len(sels[e]) - offs[e])
            take = max(take, 0)
            idx = sels[e][offs[e]:offs[e] + take]
            offs[e] += take
            plan[core][cls] = (e, idx)
    for e in range(E):
        assert offs[e] >= len(sels[e]), (e, offs[e], len(sels[e]), sizes, asg)
    return sizes, plan


def _ensure_ntff_hook():
    """Register the axon NTFF profile hook if the image lacks antenv.axon_hooks."""
    import sys
    import types
    try:
        from antenv.axon_hooks import get_axon_ntff_profile_hook
        return get_axon_ntff_profile_hook() is not None
    except ImportError:
        pass
    try:
        import antenv
        from trn_agent_boot.trn_boot import _ntff_profile_via_ctypes
        mod = types.ModuleType("antenv.axon_hooks")
        holder = [None]
        mod.set_axon_ntff_profile_hook = lambda h: holder.__setitem__(0, h)
        mod.get_axon_ntff_profile_hook = lambda: holder[0]
        sys.modules["antenv.axon_hooks"] = mod
        antenv.axon_hooks = mod
        mod.set_axon_ntff_profile_hook(
            _ntff_profile_via_ctypes("/opt/axon/libaxon_pjrt.so"))
        return True
    except Exception:
        return False


def _build(sizes):
    import concourse.mybir as mybir
    import concourse.tile as tile
    from concourse import bacc

    nc = bacc.Bacc(None, target_bir_lowering=False)

    Csum = sum(sizes)
    offs = [0, sizes[0], sizes[0] + sizes[1]]
    # GEMM1 runs a leader schedule: the biggest slot starts solo (smallest
    # dependency footprint per PE-second), the others join as their x lands.
    # GEMM2 emits big->mid->small so the final store (the tail) is smallest.
    g2_order = sorted(range(S), key=lambda s: -sizes[s])
    lead, mid_s, small_s = g2_order
    s0 = lead

    xs = []
    for s in range(S):
        if s == s0:
            # first GEMM1 slot: split along k so the first half gates less DMA
            xs.append(nc.declare_dram_parameter(
                f"x{s}", [2, P, KD // 2, sizes[s]], mybir.dt.bfloat16,
                isOutput=False))
        else:
            xs.append(nc.declare_dram_parameter(
                f"x{s}", [P, KD, sizes[s]], mybir.dt.bfloat16, isOutput=False))
    w1p, w2p, b1p, b2p = [], [], [], []
    for s in range(S):
        w1p.append([nc.declare_dram_parameter(
            f"w1_{s}_{ci}", [P, ch, KD, P], mybir.dt.bfloat16,
            isOutput=False) for ci, ch in enumerate(CHUNK_PLAN)])
        w2p.append(nc.declare_dram_parameter(
            f"w2_{s}", [KD, P, MH, P], mybir.dt.bfloat16, isOutput=False))
    biasp = nc.declare_dram_parameter(
        "bias", [P, S * (MH + KD)], mybir.dt.float32, isOutput=False)
    out = nc.declare_dram_parameter("out", [P, KD, Csum], mybir.dt.float32,
                                    isOutput=True)

    GELU = mybir.ActivationFunctionType.Gelu

    with tile.TileContext(nc) as tc, \
         tc.tile_pool(name="singles", bufs=1) as singles, \
         tc.tile_pool(name="w1pool0", bufs=4) as w1pool0, \
         tc.tile_pool(name="w1pool1", bufs=4) as w1pool1, \
         tc.tile_pool(name="w1pool2", bufs=3) as w1pool2, \
         tc.tile_pool(name="w2pool0", bufs=2) as w2pool0, \
         tc.tile_pool(name="w2pool1", bufs=2) as w2pool1, \
         tc.tile_pool(name="w2pool2", bufs=2) as w2pool2, \
         tc.tile_pool(name="ypool", bufs=3) as ypool, \
         tc.tile_pool(name="psum", bufs=4, space="PSUM") as psum_pool:
        # leader gets the deepest ring (4 chunks in flight pre-stream);
        # mid 4; small 3.
        w1pools = {lead: w1pool0, mid_s: w1pool1, small_s: w1pool2}
        w2pools = [w2pool0, w2pool1, w2pool2]

        # PE warm-up: dependency-free matmuls release the HAM clock gate
        # (2.4 GHz) while the DMA ring boots and the first x/w1 land.
        warm_sb = singles.tile([P, 2 * P], mybir.dt.bfloat16)
        nc.vector.memset(warm_sb[:], 0.0)
        ps_warm = psum_pool.tile([P, 2 * P], mybir.dt.float32, name="ps_warm",
                                 tag="ps1")
        for _ in range(18):
            nc.tensor.matmul(ps_warm[:], warm_sb[:, :P], warm_sb[:],
                             start=True, stop=True)

        # --- startup DMA, issue order = first-dependency order ---
        w1_t = {s: {} for s in range(S)}   # slot -> {chunk_index: tile}
        def load_w1_chunk(s, ci):
            ch = CHUNK_PLAN[ci]
            t = w1pools[s].tile([P, ch, KD, P], mybir.dt.bfloat16,
                                name=f"w1t{s}")
            nc.sync.dma_start(out=t[:], in_=w1p[s][ci][:])
            w1_t[s][ci] = t

        # Leader-first startup: only the leader's x + first w1 chunk + bias
        # gate the first PSUM group (~1.4MB); everything else streams behind.
        x_sb = {}
        x_half = {}
        next_ci = {s: 0 for s in range(S)}
        def issue_chunks(s, upto):
            while next_ci[s] <= min(upto, len(CHUNK_PLAN) - 1):
                load_w1_chunk(s, next_ci[s])
                next_ci[s] += 1
        xh0 = singles.tile([P, KD // 2, sizes[s0]], mybir.dt.bfloat16,
                          name="xh0")
        nc.sync.dma_start(out=xh0[:], in_=xs[s0][0])
        issue_chunks(lead, 0)
        bias_sb = singles.tile([P, S * (MH + KD)], mybir.dt.float32,
                               name="bias_sb")
        nc.sync.dma_start(out=bias_sb[:], in_=biasp[:])
        b1_sb = [bias_sb[:, s * MH:(s + 1) * MH] for s in range(S)]
        b2_sb = [bias_sb[:, S * MH + s * KD:S * MH + (s + 1) * KD]
                 for s in range(S)]
        xh1 = singles.tile([P, KD // 2, sizes[s0]], mybir.dt.bfloat16,
                          name="xh1")
        nc.sync.dma_start(out=xh1[:], in_=xs[s0][1])
        x_half[s0] = (xh0, xh1)
        issue_chunks(lead, 3)
        t = singles.tile([P, KD, sizes[mid_s]], mybir.dt.bfloat16,
                         name="xsb_mid")
        nc.sync.dma_start(out=t[:], in_=xs[mid_s][:])
        x_sb[mid_s] = t
        issue_chunks(mid_s, 0)
        t = singles.tile([P, KD, sizes[small_s]], mybir.dt.bfloat16,
                         name="xsb_small")
        nc.sync.dma_start(out=t[:], in_=xs[small_s][:])
        x_sb[small_s] = t
        issue_chunks(small_s, 0)
        hT_sb = singles.tile([P, MH, Csum], mybir.dt.bfloat16)

        def xk(s, k):
            if s in x_half:
                h0, h1 = x_half[s]
                return h0[:, k, :] if k < KD // 2 else h1[:, k - KD // 2, :]
            return x_sb[s][:, k, :]

        def load_w2(s, do):
            t = w2pools[s].tile([P, MH, P], mybir.dt.bfloat16, name=f"w2t{s}")
            nc.sync.dma_start(out=t[:], in_=w2p[s][do])
            return t

        # --- GEMM1 schedule: leader solo, then +mid, then all three ---
        cmap = []
        for ci, ch in enumerate(CHUNK_PLAN):
            for j in range(ch):
                cmap.append((ci, j))
        LEAD_A, LEAD_B = 6, 5
        nxt = {s: 0 for s in range(S)}
        sched = []
        for _ in range(LEAD_A):
            sched.append(lead)
        for _ in range(LEAD_B):
            sched.append(lead)
            sched.append(mid_s)
        done = {lead: LEAD_A + LEAD_B, mid_s: LEAD_B, small_s: 0}
        while any(v < MH for v in done.values()):
            for s in (lead, mid_s, small_s):
                if done[s] < MH:
                    sched.append(s)
                    done[s] += 1
        w2_first = None
        for gi, s in enumerate(sched):
            m = nxt[s]
            nxt[s] += 1
            ci, j = cmap[m]
            issue_chunks(s, ci + 1)
            if gi == len(sched) - 18:
                # prefetch GEMM2's first weight tiles so the PE doesn't
                # stall at the GEMM1->GEMM2 transition
                w2_first = {ss: load_w2(ss, 0) for ss in g2_order}
            ns = sizes[s]
            ps1 = psum_pool.tile([P, ns], mybir.dt.float32, name="ps1")
            for k in range(KD):
                nc.tensor.matmul(ps1[:], w1_t[s][ci][:, j, k, :], xk(s, k),
                                 start=(k == 0), stop=(k == KD - 1))
            nc.scalar.activation(
                hT_sb[:, m, offs[s]:offs[s] + ns], ps1[:], GELU,
                bias=b1_sb[s][:, m:m + 1])

        # --- GEMM2: yT[do*128+p, slot_s] = w2_s[do].T @ hT_s + b2_s ---
        for do in range(KD):
            if do == 0:
                w2_t = w2_first
            else:
                w2_t = {s: load_w2(s, do) for s in g2_order}
            for s in g2_order:
                ns = sizes[s]
                ps2 = psum_pool.tile([P, ns], mybir.dt.float32, name="ps2")
                for k in range(MH):
                    nc.tensor.matmul(ps2[:], w2_t[s][:, k, :],
                                     hT_sb[:, k, offs[s]:offs[s] + ns],
                                     start=(k == 0), stop=(k == MH - 1))
                y_sb = ypool.tile([P, ns], mybir.dt.float32, name="y_sb")
                if do == KD - 1 and s == g2_order[-1]:
                    # final group: emit in two halves so the first store
                    # overlaps the second half's bias add (shorter tail)
                    h = ns // 2
                    for lo, hi in ((0, h), (h, ns)):
                        nc.vector.tensor_scalar_add(
                            y_sb[:, lo:hi], ps2[:, lo:hi],
                            b2_sb[s][:, do:do + 1])
                        nc.sync.dma_start(
                            out=out[:, do, offs[s] + lo:offs[s] + hi],
                            in_=y_sb[:, lo:hi])
                else:
                    nc.vector.tensor_scalar_add(y_sb[:], ps2[:],
                                                b2_sb[s][:, do:do + 1])
                    nc.sync.dma_start(out=out[:, do, offs[s]:offs[s] + ns],
                                      in_=y_sb[:])

    nc.compile()
    return nc


def kernel(x, gate_w, gate_b, w1, b1, w2, b2):
    global LAST_EXEC_TIME_NS
    from concourse.bass_utils import run_bass_kernel_spmd

    x = np.asarray(x)
    xf = np.ascontiguousarray(x.reshape(N, D), dtype=np.float32)

    # --- Gate (host, float64 for a stable top-2 selection) ---
    logits = xf.astype(np.float64) @ np.asarray(gate_w).astype(np.float64)
    logits += np.asarray(gate_b).astype(np.float64)
    rows = np.arange(N)
    i1 = np.argmax(logits, axis=1)
    l1 = logits[rows, i1]
    tmp = logits.copy()
    tmp[rows, i1] = -np.inf
    i2 = np.argmax(tmp, axis=1)
    l2 = tmp[rows, i2]
    e2 = np.exp(l2 - l1)          # l1 >= l2
    wa = (1.0 / (1.0 + e2)).astype(np.float32)
    wb = (e2 / (1.0 + e2)).astype(np.float32)

    # --- Dispatch (host): per-expert token lists; slot plan from solver ---
    sels, wgts = [], []
    for e in range(E):
        sel = np.where((i1 == e) | (i2 == e))[0]
        wgt = np.where(i1[sel] == e, wa[sel], wb[sel])
        sels.append(sel)
        wgts.append(wgt)
    loads = [len(s) for s in sels]
    sizes, plan = _plan(loads, sels)
    Csum = sum(sizes)
    offs = [0, sizes[0], sizes[0] + sizes[1]]
    s0 = sorted(range(S), key=lambda s: -sizes[s])[0]

    # --- Per-expert reformatted weights (shared across cores) ---
    w1a = np.asarray(w1, dtype=np.float32)
    b1a = np.asarray(b1, dtype=np.float32)
    w2a = np.asarray(w2, dtype=np.float32)
    b2a = np.asarray(b2, dtype=np.float32)
    w1_r, w2_r, b1_r, b2_r = [], [], [], []
    w1_chunks = []
    for e in range(E):
        w1m = np.ascontiguousarray(
            w1a[e].reshape(KD, P, MH, P).transpose(2, 1, 0, 3)).astype(BF16)
        # chunk plan: [P, ch*KD*P] per chunk (partition-major)
        chunks = []
        m0 = 0
        for ch in CHUNK_PLAN:
            blk = w1m[m0:m0 + ch]                      # [ch, P, KD, P]
            blk = np.ascontiguousarray(
                blk.transpose(1, 0, 2, 3).reshape(P, ch * KD * P))
            chunks.append(blk)
            m0 += ch
        w1_chunks.append(chunks)
        w2_r.append(np.ascontiguousarray(
            w2a[e].reshape(MH, P, KD, P).transpose(2, 1, 0, 3)).astype(BF16))
        b1_r.append(np.ascontiguousarray(b1a[e].reshape(MH, P).T))
        b2_r.append(np.ascontiguousarray(b2a[e].reshape(KD, P).T))

    # --- Per-core input maps ---
    in_maps = []
    for core in range(E):
        m = {}
        for s in range(S):
            e, idx = plan[core][s]
            ns = sizes[s]
            xe = np.zeros((ns, D), dtype=np.float32)
            xe[:len(idx)] = xf[idx]
            xT = xe.T.reshape(KD, P, ns).transpose(1, 0, 2).astype(BF16)
            if s == s0:
                xT = np.ascontiguousarray(
                    xT.reshape(P, 2, KD // 2, ns).transpose(1, 0, 2, 3))
            else:
                xT = np.ascontiguousarray(xT)
            m[f"x{s}"] = xT
            for ci in range(len(CHUNK_PLAN)):
                m[f"w1_{s}_{ci}"] = w1_chunks[e][ci]
            m[f"w2_{s}"] = w2_r[e]
        m["bias"] = np.ascontiguousarray(np.concatenate(
            [b1_r[plan[core][s][0]] for s in range(S)] +
            [b2_r[plan[core][s][0]] for s in range(S)], axis=1))
        in_maps.append(m)

    key = tuple(sizes)
    if key not in _cached_nc:
        _cached_nc[key] = _build(sizes)
    nc = _cached_nc[key]

    trace = os.environ.get("MOE_KERNEL_PROFILE", "0") == "1"
    if trace:
        trace = _ensure_ntff_hook()
    res = None
    for attempt in range(3):
        try:
            res = run_bass_kernel_spmd(nc, in_maps, core_ids=list(range(E)),
                                       trace=trace and attempt == 0)
            break
        except Exception:
            # Device-unrecoverable NRT errors are transient here; retry with
            # a fresh PJRT client (last attempt re-raises).
            if attempt == 2:
                raise
            try:
                import jax
                jax.clear_caches()
                jax._src.api.clear_backends()
            except Exception:
                pass
    LAST_EXEC_TIME_NS = res.exec_time_ns

    # --- Combine (host) ---
    out_acc = np.zeros((N, D), dtype=np.float32)
    for core in range(E):
        yT = np.asarray(res.results[core]["out"])       # [P, KD, Csum] f32
        for cls in range(S):
            e, idx = plan[core][cls]
            ne = len(idx)
            if ne == 0:
                continue
            y = yT[:, :, offs[cls]:offs[cls] + sizes[cls]]
            y = y.transpose(1, 0, 2).reshape(D, sizes[cls]).T   # [ns, D]
            w = np.where(i1[idx] == e, wa[idx], wb[idx])
            out_acc[idx] += w[:, None] * y[:ne]

    return out_acc.reshape(B, T, D)


# revision 22
# speedup vs baseline: 1.0016x; 1.0016x over previous
"""MoE layer (top-2 of 8 experts), load-balanced expert-parallel across 8
Trainium2 NeuronCores.

Strategy (self-contained; shapes hardcoded for B=4,T=1024,D=1024,E=8,K=2,H=4096):
  - Host: gate logits + top-2 + softmax; solve a small packing problem that
    assigns token groups to a uniform per-core slot template (S=3 slot sizes
    shared by all cores, each slot bound to one expert per core via the DRAM
    input contents).  All 8 cores run the SAME instruction stream; only the
    data (which expert's weights / which tokens) differs, so total PE columns
    per core drops from max-expert-load (1129 for the seed-0 input) to
    sum(slot_sizes) (~1033) -- near the perfect-balance floor of 1024.
  - Device, SPMD over 8 cores: per slot s a transposed-layout FFN
      hT_s = gelu(w1_s.T @ xT_s + b1_s)   [H, N_s]
      yT_s = w2_s.T @ hT_s + b2_s         [D, N_s]
    bf16 matmul inputs, f32 PSUM accumulation.  Slot sizes are all in
    [260, 512]: >=257 keeps LDWEIGHTS hidden under the matmul stream,
    <=512 keeps one PSUM bank per accumulation group.
  - w1 is streamed in multi-m-tile chunks (8KB/partition descriptors) to
    keep the DMA descriptor count flat despite 3x weight traffic.
"""

import os

import numpy as np
import ml_dtypes

B, T, D = 4, 1024, 1024
E, K, H = 8, 2, 4 * 1024
N = B * T
P = 128
KD = D // P           # 8  k-tiles in GEMM1 / output d-tiles in GEMM2
MH = H // P           # 32 m-tiles in GEMM1 / k-tiles in GEMM2
S = 3                 # slots per core
BF16 = ml_dtypes.bfloat16

# w1 chunk plan (m-tiles per DMA): small first chunks so the first PSUM
# group's dependencies land early; 4-tile (8KB/partition) chunks after.
CHUNK_PLAN = [1, 1, 3, 3, 3, 3, 3, 3, 3, 3, 3, 3]
assert sum(CHUNK_PLAN) == MH

LAST_EXEC_TIME_NS = None
_cached_nc = {}


def _solve_template(loads):
    """Pick slot sizes (a>=b>=c, even, in [258,512]) minimizing
    a+b+c such that the 8 copies of each size can cover every expert's load.

    Returns (sizes, n) where n[e] = (n_a, n_b, n_c) slot counts for expert e.
    """
    import math
    loads = [int(x) for x in loads]
    nE = len(loads)

    def feasible(a, b, c):
        sizes = (a, b, c)
        opts = []
        for Le in loads:
            cand = []
            for na in range(0, 9):
                for nb in range(0, 9):
                    rem = Le - na * a - nb * b
                    ncc = max(0, math.ceil(rem / c)) if rem > 0 else 0
                    if ncc > 8:
                        continue
                    waste = na * a + nb * b + ncc * c - Le
                    if waste > 3 * 512:
                        continue
                    cand.append((na, nb, ncc, waste))
            if not cand:
                return None
            cand.sort(key=lambda t: t[3])
            opts.append(cand[:48])
        states = {(0, 0, 0): []}
        for cand in opts:
            new = {}
            for (ua, ub, uc), asg in states.items():
                for (na, nb, ncc, w) in cand:
                    k = (ua + na, ub + nb, uc + ncc)
                    if k[0] > 8 or k[1] > 8 or k[2] > 8:
                        continue
                    if k not in new:
                        new[k] = asg + [(na, nb, ncc)]
            states = new
            if not states:
                return None
        for asg in states.values():
            return asg
        return None

    for Tt in range(1024, 1540, 2):
        for c in range(258, Tt // 3 + 1, 2):
            for bb in range(c, (Tt - c) // 2 + 1, 2):
                a = Tt - bb - c
                if a < bb or a > 512:
                    continue
                asg = feasible(a, bb, c)
                if asg:
                    return (a, bb, c), asg
    # Fallback: one expert per core, 3 equal slots covering max load.
    Cmax = max(max(loads), 3 * 260)
    s = -(-Cmax // 12) * 4
    sizes = (s, s, s)
    asg = [(1, 1, 1) for _ in range(nE)]
    return sizes, asg


def _plan(loads, sels):
    """Build per-core slot plan: core -> [(expert, token_idx_array), ...]."""
    sizes, asg = _solve_template(loads)
    # slot lists per size class: experts with multiplicity
    slot_experts = [[], [], []]
    for e in range(E):
        for cls in range(3):
            slot_experts[cls] += [e] * asg[e][cls]
    for cls in range(3):
        while len(slot_experts[cls]) < E:
            slot_experts[cls].append(0)   # unused slot, zero tokens
        slot_experts[cls] = slot_experts[cls][:E]
    # distribute each expert's tokens across its slots greedily
    offs = [0] * E
    plan = [[None] * S for _ in range(E)]
    for cls in range(3):
        sz = sizes[cls]
        for core in range(E):
            e = slot_experts[cls][core]
            take = min(sz, len(sels[e]) - offs[e])
            take = max(take, 0)
            idx = sels[e][offs[e]:offs[e] + take]
            offs[e] += take
            plan[core][cls] = (e, idx)
    for e in range(E):
        assert offs[e] >= len(sels[e]), (e, offs[e], len(sels[e]), sizes, asg)
    return sizes, plan


def _ensure_ntff_hook():
    """Register the axon NTFF profile hook if the image lacks antenv.axon_hooks."""
    import sys
    import types
    try:
        from antenv.axon_hooks import get_axon_ntff_profile_hook
        return get_axon_ntff_profile_hook() is not None
    except ImportError:
        pass
    try:
        import antenv
        from trn_agent_boot.trn_boot import _ntff_profile_via_ctypes
        mod = types.ModuleType("antenv.axon_hooks")
        holder = [None]
        mod.set_axon_ntff_profile_hook = lambda h: holder.__setitem__(0, h)
        mod.get_axon_ntff_profile_hook = lambda: holder[0]
        sys.modules["antenv.axon_hooks"] = mod
        antenv.axon_hooks = mod
        mod.set_axon_ntff_profile_hook(
            _ntff_profile_via_ctypes("/opt/axon/libaxon_pjrt.so"))
        return True
    except Exception:
        return False


def _build(sizes):
    import concourse.mybir as mybir
    import concourse.tile as tile
    from concourse import bacc

    nc = bacc.Bacc(None, target_bir_lowering=False)

    Csum = sum(sizes)
    offs = [0, sizes[0], sizes[0] + sizes[1]]
    # GEMM1 runs a leader schedule: the biggest slot starts solo (smallest
    # dependency footprint per PE-second), the others join as their x lands.
    # GEMM2 emits big->mid->small so the final store (the tail) is smallest.
    g2_order = sorted(range(S), key=lambda s: -sizes[s])
    lead, mid_s, small_s = g2_order
    s0 = lead

    xs = []
    for s in range(S):
        if s == s0:
            # first GEMM1 slot: split along k so the first half gates less DMA
            xs.append(nc.declare_dram_parameter(
                f"x{s}", [2, P, KD // 2, sizes[s]], mybir.dt.bfloat16,
                isOutput=False))
        else:
            xs.append(nc.declare_dram_parameter(
                f"x{s}", [P, KD, sizes[s]], mybir.dt.bfloat16, isOutput=False))
    w1p, w2p, b1p, b2p = [], [], [], []
    for s in range(S):
        w1p.append([nc.declare_dram_parameter(
            f"w1_{s}_{ci}", [P, ch, KD, P], mybir.dt.bfloat16,
            isOutput=False) for ci, ch in enumerate(CHUNK_PLAN)])
        w2p.append(nc.declare_dram_parameter(
            f"w2_{s}", [KD, P, MH, P], mybir.dt.bfloat16, isOutput=False))
    biasp = nc.declare_dram_parameter(
        "bias", [P, S * (MH + KD)], mybir.dt.float32, isOutput=False)
    out = nc.declare_dram_parameter("out", [P, KD, Csum], mybir.dt.float32,
                                    isOutput=True)

    GELU = mybir.ActivationFunctionType.Gelu

    with tile.TileContext(nc) as tc, \
         tc.tile_pool(name="singles", bufs=1) as singles, \
         tc.tile_pool(name="w1pool0", bufs=4) as w1pool0, \
         tc.tile_pool(name="w1pool1", bufs=4) as w1pool1, \
         tc.tile_pool(name="w1pool2", bufs=3) as w1pool2, \
         tc.tile_pool(name="w2pool0", bufs=2) as w2pool0, \
         tc.tile_pool(name="w2pool1", bufs=2) as w2pool1, \
         tc.tile_pool(name="w2pool2", bufs=2) as w2pool2, \
         tc.tile_pool(name="ypool", bufs=3) as ypool, \
         tc.tile_pool(name="psum", bufs=4, space="PSUM") as psum_pool:
        # leader gets the deepest ring (4 chunks in flight pre-stream);
        # mid 4; small 3.
        w1pools = {lead: w1pool0, mid_s: w1pool1, small_s: w1pool2}
        w2pools = [w2pool0, w2pool1, w2pool2]

        # PE warm-up: dependency-free matmuls release the HAM clock gate
        # (2.4 GHz) while the DMA ring boots and the first x/w1 land.
        warm_sb = singles.tile([P, 2 * P], mybir.dt.bfloat16)
        nc.vector.memset(warm_sb[:], 0.0)
        ps_warm = psum_pool.tile([P, 2 * P], mybir.dt.float32, name="ps_warm",
                                 tag="ps1")
        for _ in range(20):
            nc.tensor.matmul(ps_warm[:], warm_sb[:, :P], warm_sb[:],
                             start=True, stop=True)

        # --- startup DMA, issue order = first-dependency order ---
        w1_t = {s: {} for s in range(S)}   # slot -> {chunk_index: tile}
        def load_w1_chunk(s, ci):
            ch = CHUNK_PLAN[ci]
            t = w1pools[s].tile([P, ch, KD, P], mybir.dt.bfloat16,
                                name=f"w1t{s}")
            nc.sync.dma_start(out=t[:], in_=w1p[s][ci][:])
            w1_t[s][ci] = t

        # Leader-first startup: only the leader's x + first w1 chunk + bias
        # gate the first PSUM group (~1.4MB); everything else streams behind.
        x_sb = {}
        x_half = {}
        next_ci = {s: 0 for s in range(S)}
        def issue_chunks(s, upto):
            while next_ci[s] <= min(upto, len(CHUNK_PLAN) - 1):
                load_w1_chunk(s, next_ci[s])
                next_ci[s] += 1
        xh0 = singles.tile([P, KD // 2, sizes[s0]], mybir.dt.bfloat16,
                          name="xh0")
        nc.sync.dma_start(out=xh0[:], in_=xs[s0][0])
        issue_chunks(lead, 0)
        bias_sb = singles.tile([P, S * (MH + KD)], mybir.dt.float32,
                               name="bias_sb")
        nc.sync.dma_start(out=bias_sb[:], in_=biasp[:])
        b1_sb = [bias_sb[:, s * MH:(s + 1) * MH] for s in range(S)]
        b2_sb = [bias_sb[:, S * MH + s * KD:S * MH + (s + 1) * KD]
                 for s in range(S)]
        xh1 = singles.tile([P, KD // 2, sizes[s0]], mybir.dt.bfloat16,
                          name="xh1")
        nc.sync.dma_start(out=xh1[:], in_=xs[s0][1])
        x_half[s0] = (xh0, xh1)
        issue_chunks(lead, 3)
        t = singles.tile([P, KD, sizes[mid_s]], mybir.dt.bfloat16,
                         name="xsb_mid")
        nc.sync.dma_start(out=t[:], in_=xs[mid_s][:])
        x_sb[mid_s] = t
        issue_chunks(mid_s, 0)
        t = singles.tile([P, KD, sizes[small_s]], mybir.dt.bfloat16,
                         name="xsb_small")
        nc.sync.dma_start(out=t[:], in_=xs[small_s][:])
        x_sb[small_s] = t
        issue_chunks(small_s, 0)
        hT_sb = singles.tile([P, MH, Csum], mybir.dt.bfloat16)

        def xk(s, k):
            if s in x_half:
                h0, h1 = x_half[s]
                return h0[:, k, :] if k < KD // 2 else h1[:, k - KD // 2, :]
            return x_sb[s][:, k, :]

        def load_w2(s, do):
            t = w2pools[s].tile([P, MH, P], mybir.dt.bfloat16, name=f"w2t{s}")
            nc.sync.dma_start(out=t[:], in_=w2p[s][do])
            return t

        # --- GEMM1 schedule: leader solo, then +mid, then all three ---
        cmap = []
        for ci, ch in enumerate(CHUNK_PLAN):
            for j in range(ch):
                cmap.append((ci, j))
        LEAD_A, LEAD_B = 6, 5
        nxt = {s: 0 for s in range(S)}
        sched = []
        for _ in range(LEAD_A):
            sched.append(lead)
        for _ in range(LEAD_B):
            sched.append(lead)
            sched.append(mid_s)
        done = {lead: LEAD_A + LEAD_B, mid_s: LEAD_B, small_s: 0}
        while any(v < MH for v in done.values()):
            for s in (lead, mid_s, small_s):
                if done[s] < MH:
                    sched.append(s)
                    done[s] += 1
        w2_first = None
        for gi, s in enumerate(sched):
            m = nxt[s]
            nxt[s] += 1
            ci, j = cmap[m]
            issue_chunks(s, ci + 1)
            if gi == len(sched) - 18:
                # prefetch GEMM2's first weight tiles so the PE doesn't
                # stall at the GEMM1->GEMM2 transition
                w2_first = {ss: load_w2(ss, 0) for ss in g2_order}
            ns = sizes[s]
            ps1 = psum_pool.tile([P, ns], mybir.dt.float32, name="ps1")
            for k in range(KD):
                nc.tensor.matmul(ps1[:], w1_t[s][ci][:, j, k, :], xk(s, k),
                                 start=(k == 0), stop=(k == KD - 1))
            nc.scalar.activation(
                hT_sb[:, m, offs[s]:offs[s] + ns], ps1[:], GELU,
                bias=b1_sb[s][:, m:m + 1])

        # --- GEMM2: yT[do*128+p, slot_s] = w2_s[do].T @ hT_s + b2_s ---
        for do in range(KD):
            if do == 0:
                w2_t = w2_first
            else:
                w2_t = {s: load_w2(s, do) for s in g2_order}
            for s in g2_order:
                ns = sizes[s]
                ps2 = psum_pool.tile([P, ns], mybir.dt.float32, name="ps2")
                for k in range(MH):
                    nc.tensor.matmul(ps2[:], w2_t[s][:, k, :],
                                     hT_sb[:, k, offs[s]:offs[s] + ns],
                                     start=(k == 0), stop=(k == MH - 1))
                y_sb = ypool.tile([P, ns], mybir.dt.float32, name="y_sb")
                nc.vector.tensor_scalar_add(y_sb[:], ps2[:],
                                            b2_sb[s][:, do:do + 1])
                nc.sync.dma_start(out=out[:, do, offs[s]:offs[s] + ns],
                                  in_=y_sb[:])

    nc.compile()
    return nc


def kernel(x, gate_w, gate_b, w1, b1, w2, b2):
    global LAST_EXEC_TIME_NS
    from concourse.bass_utils import run_bass_kernel_spmd

    x = np.asarray(x)
    xf = np.ascontiguousarray(x.reshape(N, D), dtype=np.float32)

    # --- Gate (host, float64 for a stable top-2 selection) ---
    logits = xf.astype(np.float64) @ np.asarray(gate_w).astype(np.float64)
    logits += np.asarray(gate_b).astype(np.float64)
    rows = np.arange(N)
    i1 = np.argmax(logits, axis=1)
    l1 = logits[rows, i1]
    tmp = logits.copy()
    tmp[rows, i1] = -np.inf
    i2 = np.argmax(tmp, axis=1)
    l2 = tmp[rows, i2]
    e2 = np.exp(l2 - l1)          # l1 >= l2
    wa = (1.0 / (1.0 + e2)).astype(np.float32)
    wb = (e2 / (1.0 + e2)).astype(np.float32)

    # --- Dispatch (host): per-expert token lists; slot plan from solver ---
    sels, wgts = [], []
    for e in range(E):
        sel = np.where((i1 == e) | (i2 == e))[0]
        wgt = np.where(i1[sel] == e, wa[sel], wb[sel])
        sels.append(sel)
        wgts.append(wgt)
    loads = [len(s) for s in sels]
    sizes, plan = _plan(loads, sels)
    Csum = sum(sizes)
    offs = [0, sizes[0], sizes[0] + sizes[1]]
    s0 = sorted(range(S), key=lambda s: -sizes[s])[0]

    # --- Per-expert reformatted weights (shared across cores) ---
    w1a = np.asarray(w1, dtype=np.float32)
    b1a = np.asarray(b1, dtype=np.float32)
    w2a = np.asarray(w2, dtype=np.float32)
    b2a = np.asarray(b2, dtype=np.float32)
    w1_r, w2_r, b1_r, b2_r = [], [], [], []
    w1_chunks = []
    for e in range(E):
        w1m = np.ascontiguousarray(
            w1a[e].reshape(KD, P, MH, P).transpose(2, 1, 0, 3)).astype(BF16)
        # chunk plan: [P, ch*KD*P] per chunk (partition-major)
        chunks = []
        m0 = 0
        for ch in CHUNK_PLAN:
            blk = w1m[m0:m0 + ch]                      # [ch, P, KD, P]
            blk = np.ascontiguousarray(
                blk.transpose(1, 0, 2, 3).reshape(P, ch * KD * P))
            chunks.append(blk)
            m0 += ch
        w1_chunks.append(chunks)
        w2_r.append(np.ascontiguousarray(
            w2a[e].reshape(MH, P, KD, P).transpose(2, 1, 0, 3)).astype(BF16))
        b1_r.append(np.ascontiguousarray(b1a[e].reshape(MH, P).T))
        b2_r.append(np.ascontiguousarray(b2a[e].reshape(KD, P).T))

    # --- Per-core input maps ---
    in_maps = []
    for core in range(E):
        m = {}
        for s in range(S):
            e, idx = plan[core][s]
            ns = sizes[s]
            xe = np.zeros((ns, D), dtype=np.float32)
            xe[:len(idx)] = xf[idx]
            xT = xe.T.reshape(KD, P, ns).transpose(1, 0, 2).astype(BF16)
            if s == s0:
                xT = np.ascontiguousarray(
                    xT.reshape(P, 2, KD // 2, ns).transpose(1, 0, 2, 3))
            else:
                xT = np.ascontiguousarray(xT)
            m[f"x{s}"] = xT
            for ci in range(len(CHUNK_PLAN)):
                m[f"w1_{s}_{ci}"] = w1_chunks[e][ci]
            m[f"w2_{s}"] = w2_r[e]
        m["bias"] = np.ascontiguousarray(np.concatenate(
            [b1_r[plan[core][s][0]] for s in range(S)] +
            [b2_r[plan[core][s][0]] for s in range(S)], axis=1))
        in_maps.append(m)

    key = tuple(sizes)
    if key not in _cached_nc:
        _cached_nc[key] = _build(sizes)
    nc = _cached_nc[key]

    trace = os.environ.get("MOE_KERNEL_PROFILE", "0") == "1"
    if trace:
        trace = _ensure_ntff_hook()
    res = None
    for attempt in range(3):
        try:
            res = run_bass_kernel_spmd(nc, in_maps, core_ids=list(range(E)),
                                       trace=trace and attempt == 0)
            break
        except Exception:
            # Device-unrecoverable NRT errors are transient here; retry with
            # a fresh PJRT client (last attempt re-raises).
            if attempt == 2:
                raise
            try:
                import jax
                jax.clear_caches()
                jax._src.api.clear_backends()
            except Exception:
                pass
    LAST_EXEC_TIME_NS = res.exec_time_ns

    # --- Combine (host) ---
    out_acc = np.zeros((N, D), dtype=np.float32)
    for core in range(E):
        yT = np.asarray(res.results[core]["out"])       # [P, KD, Csum] f32
        for cls in range(S):
            e, idx = plan[core][cls]
            ne = len(idx)
            if ne == 0:
                continue
            y = yT[:, :, offs[cls]:offs[cls] + sizes[cls]]
            y = y.transpose(1, 0, 2).reshape(D, sizes[cls]).T   # [ns, D]
            w = np.where(i1[idx] == e, wa[idx], wb[idx])
            out_acc[idx] += w[:, None] * y[:ne]

    return out_acc.reshape(B, T, D)
